# revision 1
# baseline (speedup 1.0000x reference)
"""Bass/Trainium2 kernel for nn_GroundingLoss (symmetric token-level InfoNCE).

Math (matches the jax reference exactly):
    sim[a,b,i,j] = sum_k x[a,i,k] * z[b,j,k]
    S[a,b]       = (1/J) * sum_j  [ sum_i softmax_i(sim[a,b,:,j]) * sim[a,b,:,j] ]
    loss         = mean( logsumexp_a(S) - diag + logsumexp_b(S) - diag )

Sharding: the batch axis of x (a) is split across the 8 cores; every core
computes S[a_local, :] against all of z.

Device layout per core (v2): partitions = (a_sub, i) per a-tile (4 a's x 32
i's = 128), free = (b, j) in chunks of 1024 (32 b's x 32 j's, 8 chunks).
Per (chunk, atile) the PE computes sim = xT_t.T @ zT_c into PSUM (bf16
inputs, fp32 accum), ACT computes e = exp(sim - SHIFT) -> SBUF bf16, DVE
computes es = e * sim -> SBUF bf16.  The i-reductions (num = sum_i es,
den = sum_i e) are block-diagonal ones-matmuls on the PE accumulating over
all 8 a-tiles into PSUM [32, 1024] (DVE tensor_reduce has no fast mode on
TRN2, so reductions live on the PE instead).  Per chunk the DVE finishes:
r = num * recip(den), jsum = sum_j r -> out [32, 256].  The host divides by
J, assembles S, and does the tiny [256,256] logsumexp epilogue (softmax
weights are shift-invariant, so no SHIFT correction is needed).
"""

import numpy as np

N, I, J, K = 256, 32, 32, 256
NCORES = 8
NL = N // NCORES          # 32 local a's per core
AF = NL * I               # 1024 xt cols per K-half (a, i)
BJ = N * J                # 8192 (b, j) pairs
BJC = 512                 # free elements per chunk (16 b's x 32 j's)
NCHUNK = BJ // BJC        # 16
NAT = NL // 4             # 8 a-tiles of (4 a's x 32 i's) = 128 partitions
SHIFT = 60.0              # exp shift: safe for |sim| up to ~130

_cached = None


def _build():
    import concourse.bacc as bacc
    import concourse.mybir as mybir
    import concourse.tile as tile

    f32 = mybir.dt.float32
    bf16 = mybir.dt.bfloat16
    AF_T = mybir.ActivationFunctionType
    AX = mybir.AxisListType

    nc = bacc.Bacc("TRN2", target_bir_lowering=False, debug=False)
    xt_d = nc.dram_tensor("xt", [128, 2 * AF], bf16, kind="ExternalInput").ap()
    zt_d = nc.dram_tensor("zt", [128, 2 * BJ], bf16, kind="ExternalInput").ap()
    on_d = nc.dram_tensor("ones", [128, NAT * NL], bf16, kind="ExternalInput").ap()
    out_d = nc.dram_tensor("out", [NL, 2 * BJ], f32, kind="ExternalOutput").ap()

    with tile.TileContext(nc) as tc:
        with (
            tc.tile_pool(name="const", bufs=1) as cpool,
            tc.tile_pool(name="psum", bufs=3, space="PSUM") as ppool,
            tc.tile_pool(name="nd", bufs=2, space="PSUM") as ndpool,
            tc.tile_pool(name="sb", bufs=6) as spool,
            tc.tile_pool(name="ob", bufs=1) as opool,
        ):
            bias_t = cpool.tile([128, 1], f32)
            nc.gpsimd.memset(bias_t[:], -SHIFT)
            xt = cpool.tile([128, 2 * AF], bf16)
            nc.sync.dma_start(xt[:], xt_d[:, :])
            ones = cpool.tile([128, NAT * NL], bf16)
            nc.sync.dma_start(ones[:], on_d[:, :])
            zt = cpool.tile([128, 2 * BJ], bf16)
            # split the 4MB z load so early chunks can start before the tail
            nq = 4
            for kc in range(2):
                for q in range(nq):
                    sl = slice(kc * BJ + q * (BJ // nq), kc * BJ + (q + 1) * (BJ // nq))
                    nc.sync.dma_start(zt[:, sl], zt_d[:, sl])

            ob = opool.tile([NL, 2 * BJ], f32)

            for c in range(NCHUNK):
                num_ps = ndpool.tile([NL, BJC], f32, tag="num")
                den_ps = ndpool.tile([NL, BJC], f32, tag="den")
                for t in range(NAT):
                    sim = ppool.tile([128, BJC], f32, tag="sim")
                    for kc in range(2):
                        lhsT = xt[:, kc * AF + t * 128 : kc * AF + (t + 1) * 128]
                        rhs = zt[:, kc * BJ + c * BJC : kc * BJ + (c + 1) * BJC]
                        nc.tensor.matmul(
                            sim[:], lhsT, rhs, start=(kc == 0), stop=(kc == 1)
                        )

                    e = spool.tile([128, BJC], bf16, tag="e")
                    nc.scalar.activation(e[:], sim[:], AF_T.Exp, bias=bias_t[:], scale=1.0)
                    es = spool.tile([128, BJC], bf16, tag="es")
                    nc.vector.tensor_mul(es[:], e[:], sim[:])

                    onesT = ones[:, t * NL : (t + 1) * NL]
                    nc.tensor.matmul(
                        num_ps[:], onesT, es[:],
                        start=(t == 0), stop=(t == NAT - 1),
                    )
                    nc.tensor.matmul(
                        den_ps[:], onesT, e[:],
                        start=(t == 0), stop=(t == NAT - 1),
                    )

                # ship num/den to the host (device division + j-sum stalls
                # the PE via the slow DVE reciprocal); cheap PSUM->SBUF copies
                nc.vector.tensor_copy(ob[:, 2 * c * BJC : 2 * c * BJC + BJC], num_ps[:])
                nc.scalar.activation(
                    ob[:, 2 * c * BJC + BJC : 2 * (c + 1) * BJC], den_ps[:], AF_T.Copy
                )
            nc.sync.dma_start(out_d[:, :], ob[:])
    nc.compile()
    return nc


def _prep_inputs(x, z):
    import ml_dtypes

    bf = ml_dtypes.bfloat16
    x = np.ascontiguousarray(x, dtype=np.float32).astype(bf)
    z = np.ascontiguousarray(z, dtype=np.float32).astype(bf)
    # zT[p, kc*BJ + b*J + j] = z[b, j, kc*128 + p]
    zt = z.transpose(2, 0, 1).reshape(K, BJ)
    zt = np.concatenate([zt[0:128], zt[128:256]], axis=1)
    zt = np.ascontiguousarray(zt)
    # block-diagonal ones: ones[p, t*NL + 4t + p//32] = 1
    on = np.zeros((128, NAT * NL), dtype=bf)
    for t in range(NAT):
        for p in range(128):
            on[p, t * NL + 4 * t + p // 32] = 1
    in_maps = []
    for d in range(NCORES):
        xl = x[d * NL : (d + 1) * NL]                  # [NL, I, K]
        xt = xl.transpose(2, 0, 1).reshape(K, AF)      # [K, (a,i)]
        xt = np.concatenate([xt[0:128], xt[128:256]], axis=1)
        in_maps.append({"xt": np.ascontiguousarray(xt), "zt": zt, "ones": on})
    return in_maps


def _epilogue(results):
    S = np.empty((N, N), dtype=np.float64)
    for d in range(NCORES):
        arr = results[d]["out"].astype(np.float64).reshape(NL, NCHUNK, 2, BJC)
        r = arr[:, :, 0, :] / arr[:, :, 1, :]          # [NL, chunk, (b,j)]
        r = r.reshape(NL, NCHUNK, BJC // J, J).mean(axis=3)
        S[d * NL : (d + 1) * NL, :] = r.reshape(NL, N)
    diag = np.diagonal(S)
    m0 = S.max(axis=0)
    lx = m0 + np.log(np.exp(S - m0[None, :]).sum(axis=0)) - diag
    m1 = S.max(axis=1)
    lz = m1 + np.log(np.exp(S - m1[:, None]).sum(axis=1)) - diag
    loss = (lx + lz).mean()
    return np.asarray(loss, dtype=np.float32)


def run_on_device(x, z, trace=False):
    """Returns (loss, BassKernelResults)."""
    from concourse.bass_utils import run_bass_kernel_spmd

    global _cached
    if _cached is None:
        _cached = _build()
    nc = _cached
    in_maps = _prep_inputs(x, z)
    res = run_bass_kernel_spmd(nc, in_maps, list(range(NCORES)), trace=trace)
    return _epilogue(res.results), res


def kernel(x, z):
    loss, _ = run_on_device(x, z)
    return loss

